# revision 1
# baseline (speedup 1.0000x reference)
"""APPNP GNN (GCN -> 10x APPNP -> GCN) on 8 TRN2 NeuronCores.

Math refactoring (exact, linear algebra):
  Reference: P = D^-1/2 (A+I) D^-1/2  (on 800k random edges + self loops)
     h = P(x@W1+b1);  h_{k+1} = 0.9*P*h_k + 0.1*h_0 (10 iters);  out = P(h@W3+b3)
  Since right-multiplication commutes with P, fold W3 in early (W13 = W1@W3):
     g_0 = P(x@W13 + b13);  g_{k+1} = 0.9*P*g_k + 0.1*g_0;  out = P*g_K + r*b3^T
  with b13 = b1@W3, r = P@1.  Feature dim drops 128 -> 64 for all propagations.
  Substituting u_k = D^-1/2 g_k makes the inner op a *plain* adjacency sum:
     z   = D^-1/2 (x@W13 + b13)
     u_0 = D^-1 B z                  (B = A+I, unweighted 0/1)
     u_{k+1} = 0.9 D^-1 B u_k + 0.1 u_0
     out = D^-1/2 B u_10 + r b3^T
  so the per-edge weights disappear; only per-row scalings remain.

Device strategy (per core, rows sharded 8 ways):
  - 12 rounds of y = B u: dma_gather rows of u (f32, 64 feats = 256B) by edge
    src index from a replicated DRAM table, segment-sum into 128-row output
    tiles via TensorE matmuls with 0/1 selector matrices (built on DVE with
    is_equal against an iota), then AllGather the new shard into the table.
  - Edge lists are preprocessed on host (graph structure only): sorted by dst
    tile, split into lo/hi halves (int16 gather index limit), padded so all 8
    cores share one SPMD program structure.
"""

import math

import numpy as np

# ---------------- problem constants (hardcoded; kernel.py is standalone) ----
N_NODES = 50000
F_IN = 256
F_MID = 128
F_OUT = 64
ALPHA = 0.1
K_ITERS = 10
N_CORES = 8
P = 128
HALF_LIM = 32768  # int16 gather index limit

DEFAULT_CFG = dict(
    n_nodes=N_NODES,
    f_in=F_IN,
    f_out=F_OUT,
    n_cores=N_CORES,
    tiles_per_group=7,
    n_rounds=K_ITERS + 2,
)


def _derive(cfg):
    n = cfg["n_nodes"]
    nc_ = cfg["n_cores"]
    assert n % nc_ == 0
    rpc = n // nc_
    tiles = math.ceil(rpc / P)
    tpg = cfg["tiles_per_group"]
    assert tiles % tpg == 0, (tiles, tpg)
    groups = tiles // tpg
    half = n // 2
    assert half <= HALF_LIM and (n - half) <= HALF_LIM
    return rpc, tiles, tpg, groups, half


# ---------------------------------------------------------------- host prep
def preprocess(edge_index, cfg):
    """Graph-structure preprocessing (indices/degrees only).

    Returns (meta, per_core) where meta is identical structural info for the
    single SPMD program and per_core[c] holds that core's data arrays.
    """
    n = cfg["n_nodes"]
    rpc, tiles, tpg, groups, half = _derive(cfg)

    src = np.asarray(edge_index[0], dtype=np.int64)
    dst = np.asarray(edge_index[1], dtype=np.int64)
    loops = np.arange(n, dtype=np.int64)
    src = np.concatenate([src, loops])
    dst = np.concatenate([dst, loops])

    deg = np.bincount(dst, minlength=n).astype(np.float64)  # >= 1 (self loops)
    dinv = 1.0 / np.sqrt(deg)
    rvec = dinv * np.bincount(dst, weights=dinv[src], minlength=n)

    core_of = dst // rpc
    local = dst - core_of * rpc
    tile_of = np.minimum(local // P, tiles - 1)
    islo = src < half

    # per (core, tile, half) counts -> shared structural chunk counts
    key = (core_of * tiles + tile_of) * 2 + (~islo).astype(np.int64)
    cnt = np.bincount(key, minlength=cfg["n_cores"] * tiles * 2).reshape(
        cfg["n_cores"], tiles, 2
    )
    CLO = [int(math.ceil(cnt[:, t, 0].max() / P)) for t in range(tiles)]
    CHI = [int(math.ceil(cnt[:, t, 1].max() / P)) for t in range(tiles)]
    # chunk bookkeeping (identical across cores)
    GLO = [sum(CLO[g * tpg : (g + 1) * tpg]) for g in range(groups)]
    GHI = [sum(CHI[g * tpg : (g + 1) * tpg]) for g in range(groups)]
    CG = [GLO[g] + GHI[g] for g in range(groups)]
    c_tot = sum(CG)
    i_tot = c_tot * P

    # sort edges by (core, local dst)
    order = np.lexsort((local, core_of))
    src_s, local_s, core_s, islo_s = (
        src[order],
        local[order],
        core_of[order],
        islo[order],
    )

    per_core = []
    core_bounds = np.searchsorted(core_s, np.arange(cfg["n_cores"] + 1))
    for c in range(cfg["n_cores"]):
        a, b = core_bounds[c], core_bounds[c + 1]
        csrc, clocal, cislo = src_s[a:b], local_s[a:b], islo_s[a:b]
        tile_bounds = np.searchsorted(clocal, np.arange(tiles + 1) * P)

        idx_list = np.zeros(i_tot, dtype=np.int16)
        dstl_flat = np.full(c_tot * P, -1.0, dtype=np.float32)
        pos = 0  # gathered-row position == chunk*128+lane order
        for g in range(groups):
            for lo_pass in (True, False):
                for t in range(g * tpg, (g + 1) * tpg):
                    ta, tb = tile_bounds[t], tile_bounds[t + 1]
                    m = cislo[ta:tb] if lo_pass else ~cislo[ta:tb]
                    s_t = csrc[ta:tb][m]
                    d_t = clocal[ta:tb][m] - t * P
                    cpad = (CLO[t] if lo_pass else CHI[t]) * P
                    assert len(s_t) <= cpad
                    idx_list[pos : pos + len(s_t)] = (
                        s_t if lo_pass else s_t - half
                    ).astype(np.int16)
                    dstl_flat[pos : pos + len(s_t)] = d_t.astype(np.float32)
                    pos += cpad
        assert pos == i_tot

        # idx wrapped across 16 partitions, replicated to all 8 groups of 16
        idx_w = idx_list.reshape(-1, 16).T.copy()  # [16, i_tot//16]
        idx_rep = np.tile(idx_w, (8, 1))  # [128, i_tot//16]
        dstl = dstl_flat.reshape(-1, P).T.copy()  # [128, c_tot]

        # per-node constant columns [128, tiles]
        nodes = c * rpc + np.arange(tiles * P)
        valid = nodes < (c + 1) * rpc
        nodes_c = np.where(valid, nodes, c * rpc)
        col = lambda v: (
            np.where(valid, v[nodes_c], 0.0).reshape(tiles, P).T.astype(np.float32)
        ).copy()
        cu0 = col(1.0 / deg)
        cmid = col((1.0 - ALPHA) / deg)
        cfin = col(dinv)
        rcol = col(rvec)

        per_core.append(
            dict(
                idx=idx_rep,
                dstl=dstl,
                cu0=cu0,
                cmid=cmid,
                cfin=cfin,
                rcol=rcol,
            )
        )

    meta = dict(
        CLO=CLO,
        CHI=CHI,
        GLO=GLO,
        GHI=GHI,
        CG=CG,
        c_tot=c_tot,
        i_tot=i_tot,
        rpc=rpc,
        tiles=tiles,
        tpg=tpg,
        groups=groups,
        half=half,
    )
    return meta, per_core


def host_inputs(x, W1, b1, W3, b3, rvec_unused, meta, per_core, cfg):
    """Per-core in_maps combining structure data with weight/feature data."""
    W13 = (np.asarray(W1, np.float64) @ np.asarray(W3, np.float64)).astype(np.float32)
    b13 = (np.asarray(b1, np.float64) @ np.asarray(W3, np.float64)).astype(np.float32)
    b3 = np.asarray(b3, np.float32)
    rpc = meta["rpc"]
    tiles = meta["tiles"]
    iota = np.broadcast_to(np.arange(P, dtype=np.float32), (P, P)).copy()
    in_maps = []
    for c, pc in enumerate(per_core):
        xT = np.ascontiguousarray(np.asarray(x, np.float32)[c * rpc : (c + 1) * rpc].T)
        rb3 = (
            (pc["rcol"].reshape(P, tiles, 1) * b3.reshape(1, 1, F_OUT))
            .reshape(P, tiles * F_OUT)
            .astype(np.float32)
        )
        in_maps.append(
            dict(
                xT=xT,
                W13=W13,
                b13=b13.reshape(F_OUT, 1).copy(),
                rb3=rb3,
                idx=pc["idx"],
                dstl=pc["dstl"],
                cu0=pc["cu0"],
                cmid=pc["cmid"],
                cfin=pc["cfin"],
                iota=iota,
            )
        )
    return in_maps


# ---------------------------------------------------------------- bass build
def build(meta, cfg):
    import os
    ABL_NO_ISEQ = bool(int(os.environ.get("ABL_NO_ISEQ", "0")))
    ABL_ONE_MM = bool(int(os.environ.get("ABL_ONE_MM", "0")))
    ABL_NO_DENSE = bool(int(os.environ.get("ABL_NO_DENSE", "0")))
    from concourse import bacc, bass, mybir, tile
    from concourse.bass import AP
    from concourse import library_config
    from concourse.masks import make_identity

    f32 = mybir.dt.float32
    i16 = mybir.dt.int16
    Copy = mybir.ActivationFunctionType.Copy
    Ident = mybir.ActivationFunctionType.Identity

    rpc = meta["rpc"]
    tiles = meta["tiles"]
    tpg = meta["tpg"]
    groups = meta["groups"]
    half = meta["half"]
    CLO, CHI, GLO, GHI, CG = (
        meta["CLO"],
        meta["CHI"],
        meta["GLO"],
        meta["GHI"],
        meta["CG"],
    )
    c_tot, i_tot = meta["c_tot"], meta["i_tot"]
    n = cfg["n_nodes"]
    f_in = cfg["f_in"]
    fo = cfg["f_out"]
    n_rounds = cfg["n_rounds"]
    cgmax = max(CG)
    ctmax = max(CLO[t] + CHI[t] for t in range(tiles))
    rg = [list(range(cfg["n_cores"]))]

    nc = bacc.Bacc(None, target_bir_lowering=False, debug=False)

    xT_p = nc.declare_dram_parameter("xT", [f_in, rpc], f32, isOutput=False)
    W13_p = nc.declare_dram_parameter("W13", [f_in, fo], f32, isOutput=False)
    b13_p = nc.declare_dram_parameter("b13", [fo, 1], f32, isOutput=False)
    rb3_p = nc.declare_dram_parameter("rb3", [P, tiles * fo], f32, isOutput=False)
    idx_p = nc.declare_dram_parameter("idx", [P, i_tot // 16], i16, isOutput=False)
    dstl_p = nc.declare_dram_parameter("dstl", [P, c_tot], f32, isOutput=False)
    cu0_p = nc.declare_dram_parameter("cu0", [P, tiles], f32, isOutput=False)
    cmid_p = nc.declare_dram_parameter("cmid", [P, tiles], f32, isOutput=False)
    cfin_p = nc.declare_dram_parameter("cfin", [P, tiles], f32, isOutput=False)
    iota_p = nc.declare_dram_parameter("iota", [P, P], f32, isOutput=False)
    out_p = nc.declare_dram_parameter("out", [rpc, fo], f32, isOutput=True)

    # internal DRAM: per-round gather tables (AllGather outs) + shard bufs
    T = [
        nc.dram_tensor(f"T{k}", [n, fo], f32, addr_space="Shared")
        for k in range(n_rounds)
    ]
    shard = [nc.dram_tensor(f"sh{k}", [rpc, fo], f32) for k in range(n_rounds)]

    with tile.TileContext(nc) as tc:
        nc.gpsimd.load_library(library_config.mlp)
        with (
            tc.tile_pool(name="const", bufs=1) as cp,
            tc.tile_pool(name="psA", bufs=2, space="PSUM") as psA,
            tc.tile_pool(name="psT", bufs=2, space="PSUM") as psT,
            tc.tile_pool(name="dense", bufs=3) as dp,
            tc.tile_pool(name="gat", bufs=2) as gp,
            tc.tile_pool(name="sel", bufs=4) as sp,
            tc.tile_pool(name="outt", bufs=4) as op,
            tc.tile_pool(name="ps2", bufs=4, space="PSUM") as pp2,
        ):
            # ---------------- resident constants ----------------
            idx_sb = cp.tile([P, i_tot // 16], i16)
            nc.sync.dma_start(idx_sb[:], idx_p[:])
            dstl_sb = cp.tile([P, c_tot], f32)
            nc.sync.dma_start(dstl_sb[:], dstl_p[:])
            cu0_sb = cp.tile([P, tiles], f32)
            nc.sync.dma_start(cu0_sb[:], cu0_p[:])
            cmid_sb = cp.tile([P, tiles], f32)
            nc.sync.dma_start(cmid_sb[:], cmid_p[:])
            cfin_sb = cp.tile([P, tiles], f32)
            nc.sync.dma_start(cfin_sb[:], cfin_p[:])
            iota_sb = cp.tile([P, P], f32)
            nc.sync.dma_start(iota_sb[:], iota_p[:])
            rb3_sb = cp.tile([P, tiles * fo], f32)
            nc.sync.dma_start(rb3_sb[:], rb3_p[:])
            b13_sb = cp.tile([fo, 1], f32)
            nc.sync.dma_start(b13_sb[:], b13_p[:])
            w13_sb = cp.tile([P, (f_in // P) * fo], f32)
            for kk in range(f_in // P):
                nc.sync.dma_start(
                    w13_sb[:, kk * fo : (kk + 1) * fo],
                    W13_p[kk * P : (kk + 1) * P, :],
                )
            ident = cp.tile([P, P], f32)
            make_identity(nc, ident[:])
            u0s_sb = cp.tile([P, tiles * fo], f32)  # 0.1 * u0, written round 0

            # ---------------- dense phase: z = dinv*(x@W13 + b13) ----------
            NBLK = 512
            nblocks = math.ceil(rpc / NBLK)
            if ABL_NO_DENSE:
                zsrc = dp.tile([P, fo], f32, tag="zz")
                nc.vector.memset(zsrc[:], 0.125)
                for t in range(tiles):
                    rows = min(P, rpc - t * P)
                    nc.sync.dma_start(shard[0][t * P : t * P + rows, :], zsrc[:rows, :])
            for bi in range(0 if ABL_NO_DENSE else nblocks):
                w = min(NBLK, rpc - bi * NBLK)
                ps = psA.tile([fo, NBLK], f32, tag="ps")
                for kk in range(f_in // P):
                    xt = dp.tile([P, NBLK], f32, tag="xt")
                    nc.sync.dma_start(
                        xt[:, :w], xT_p[kk * P : (kk + 1) * P, bi * NBLK : bi * NBLK + w]
                    )
                    nc.tensor.matmul(
                        ps[:, :w],
                        lhsT=w13_sb[:, kk * fo : (kk + 1) * fo],
                        rhs=xt[:, :w],
                        start=(kk == 0),
                        stop=(kk == f_in // P - 1),
                    )
                zt = dp.tile([fo, NBLK], f32, tag="zt")
                nc.scalar.activation(zt[:, :w], ps[:, :w], Ident, bias=b13_sb[:, :1])
                for s in range(math.ceil(w / P)):
                    sw = min(P, w - s * P)
                    tg = (bi * NBLK + s * P) // P  # global tile index
                    pt = psT.tile([P, fo], f32, tag="pt")
                    nc.tensor.transpose(
                        pt[:sw, :], zt[:, s * P : s * P + sw], ident[:fo, :fo]
                    )
                    zz = dp.tile([P, fo], f32, tag="zz")
                    nc.scalar.activation(
                        zz[:sw, :], pt[:sw, :], Copy, scale=cfin_sb[:sw, tg : tg + 1]
                    )
                    nc.sync.dma_start(
                        shard[0][bi * NBLK + s * P : bi * NBLK + s * P + sw, :],
                        zz[:sw, :],
                    )
            nc.gpsimd.collective_compute(
                "AllGather",
                mybir.AluOpType.bypass,
                replica_groups=rg,
                ins=[shard[0][:]],
                outs=[T[0][:]],
            )

            # ---------------- propagation rounds ----------------
            # idx column offsets (in idx int16 columns of 1/16th rows)
            idx_col = 0
            idx_off = []  # per (g, lo/hi) start col
            for g in range(groups):
                idx_off.append((idx_col, idx_col + GLO[g] * 8))
                idx_col += CG[g] * 8
            chunk_base = [sum(CG[:g]) for g in range(groups)]

            for rnd in range(n_rounds):
                Tin = T[rnd]
                for g in range(groups):
                    gb = gp.tile([P, cgmax * fo], f32, tag="gb")
                    lo0, hi0 = idx_off[g]
                    gv_lo = gb[:, 0 : GLO[g] * fo].rearrange(
                        "p (c e) -> p c e", e=fo
                    )
                    nc.gpsimd.dma_gather(
                        out_ap=gv_lo,
                        in_ap=Tin[0:half, :],
                        idxs_ap=idx_sb[:, lo0 : lo0 + GLO[g] * 8],
                        num_idxs=GLO[g] * P,
                        num_idxs_reg=GLO[g] * P,
                        elem_size=fo,
                        single_packet=False,
                    )
                    gv_hi = gb[:, GLO[g] * fo : CG[g] * fo].rearrange(
                        "p (c e) -> p c e", e=fo
                    )
                    nc.gpsimd.dma_gather(
                        out_ap=gv_hi,
                        in_ap=Tin[half:n, :],
                        idxs_ap=idx_sb[:, hi0 : hi0 + GHI[g] * 8],
                        num_idxs=GHI[g] * P,
                        num_idxs_reg=GHI[g] * P,
                        elem_size=fo,
                        single_packet=False,
                    )
                    lo_pref = 0
                    hi_pref = 0
                    for tl in range(tpg):
                        t = g * tpg + tl
                        rows = min(P, rpc - t * P)
                        ct = CLO[t] + CHI[t]
                        st = sp.tile([P, ctmax * P], f32, tag="st")
                        # selector build: S[p, c, r] = (dstl[p, c] == r)
                        for lo_pass in (True, False):
                            nchunk = CLO[t] if lo_pass else CHI[t]
                            if nchunk == 0 or ABL_NO_ISEQ:
                                continue
                            dcol = chunk_base[g] + (
                                lo_pref if lo_pass else GLO[g] + hi_pref
                            )
                            scol = 0 if lo_pass else CLO[t] * P
                            din = dstl_sb[:, dcol : dcol + nchunk].to_broadcast(
                                [P, nchunk, P]
                            )
                            iin = AP(
                                iota_sb[:].tensor,
                                iota_sb[:].offset,
                                [iota_sb[:].ap[0], [0, nchunk], [1, P]],
                            )
                            sout = st[:, scol : scol + nchunk * P].rearrange(
                                "p (c r) -> p c r", r=P
                            )
                            nc.vector.tensor_tensor(
                                out=sout,
                                in0=din,
                                in1=iin,
                                op=mybir.AluOpType.is_equal,
                            )
                        pt = pp2.tile([P, fo], f32, tag="pt2")
                        if ABL_NO_ISEQ:
                            nc.vector.memset(st[:, 0 : ctmax * P], 0.0)
                        for j in range(1 if ABL_ONE_MM else ct):
                            if j < CLO[t]:
                                gchunk = lo_pref + j
                            else:
                                gchunk = GLO[g] + hi_pref + (j - CLO[t])
                            nc.tensor.matmul(
                                pt[:, :],
                                lhsT=st[:, j * P : (j + 1) * P],
                                rhs=gb[:, gchunk * fo : (gchunk + 1) * fo],
                                start=(j == 0),
                                stop=(j == (0 if ABL_ONE_MM else ct - 1)),
                            )
                        lo_pref += CLO[t]
                        hi_pref += CHI[t]
                        # epilogue
                        ut = op.tile([P, fo], f32, tag="ut")
                        if rnd == 0:
                            nc.scalar.activation(
                                ut[:rows, :],
                                pt[:rows, :],
                                Copy,
                                scale=cu0_sb[:rows, t : t + 1],
                            )
                            nc.scalar.mul(
                                u0s_sb[:rows, t * fo : t * fo + fo],
                                ut[:rows, :],
                                ALPHA,
                            )
                        elif rnd < n_rounds - 1:
                            nc.scalar.activation(
                                ut[:rows, :],
                                pt[:rows, :],
                                Copy,
                                scale=cmid_sb[:rows, t : t + 1],
                            )
                            nc.vector.tensor_add(
                                ut[:rows, :],
                                ut[:rows, :],
                                u0s_sb[:rows, t * fo : t * fo + fo],
                            )
                        else:
                            nc.scalar.activation(
                                ut[:rows, :],
                                pt[:rows, :],
                                Copy,
                                scale=cfin_sb[:rows, t : t + 1],
                            )
                            nc.vector.tensor_add(
                                ut[:rows, :],
                                ut[:rows, :],
                                rb3_sb[:rows, t * fo : t * fo + fo],
                            )
                        dst_dram = out_p if rnd == n_rounds - 1 else shard[rnd + 1]
                        nc.sync.dma_start(
                            dst_dram[t * P : t * P + rows, :], ut[:rows, :]
                        )
                if rnd < n_rounds - 1:
                    nc.gpsimd.collective_compute(
                        "AllGather",
                        mybir.AluOpType.bypass,
                        replica_groups=rg,
                        ins=[shard[rnd + 1][:]],
                        outs=[T[rnd + 1][:]],
                    )
    nc.compile()
    return nc


# ---------------------------------------------------------------- runner
def run(x, edge_index, W1, b1, W3, b3, cfg=None, trace=False):
    from concourse.bass_utils import run_bass_kernel_spmd

    cfg = cfg or DEFAULT_CFG
    meta, per_core = preprocess(edge_index, cfg)
    in_maps = host_inputs(x, W1, b1, W3, b3, None, meta, per_core, cfg)
    nc = build(meta, cfg)
    res = run_bass_kernel_spmd(
        nc, in_maps, core_ids=list(range(cfg["n_cores"])), trace=trace
    )
    out = np.concatenate(
        [res.results[i]["out"] for i in range(cfg["n_cores"])], axis=0
    )
    return out.astype(np.float32), res


def kernel(**inputs):
    out, _ = run(
        inputs["x"],
        inputs["edge_index"],
        inputs["W1"],
        inputs["b1"],
        inputs["W3"],
        inputs["b3"],
    )
    return out



# revision 9
# speedup vs baseline: 1.9998x; 1.9998x over previous
"""APPNP GNN (GCN -> 10x APPNP -> GCN) on 8 TRN2 NeuronCores.

Math refactoring (exact, linear algebra):
  Reference: P = D^-1/2 (A+I) D^-1/2  (on 800k random edges + self loops)
     h = P(x@W1+b1);  h_{k+1} = 0.9*P*h_k + 0.1*h_0 (10 iters);  out = P(h@W3+b3)
  Right-multiplication commutes with P, so fold W3 in early (W13 = W1@W3):
     g_0 = P(x@W13 + b13);  g_{k+1} = 0.9*P*g_k + 0.1*g_0;  out = P*g_K + r*b3^T
  with b13 = b1@W3, r = P@1.  Feature dim drops 128 -> 64 for all propagations.
  Substituting u_k = D^-1/2 g_k makes the inner op a plain adjacency sum:
     z   = D^-1/2 (x@W13 + b13)
     u_0 = D^-1 B z                  (B = A+I, unweighted 0/1)
     u_{k+1} = 0.9 D^-1 B u_k + 0.1 u_0
     out = D^-1/2 B u_10 + r b3^T
  The self-loop (+I) term is applied as a local tile add (u_prev kept in
  SBUF), so only the 800k real edges go through the gather path.

Device strategy (per core, dst rows sharded 8 ways, 12 rounds of y = B u):
  - Node table in DRAM as bf16 [25000 pairs, 128]: node n's 64 features at
    row n>>1, halves selected by n&1. 256B rows satisfy dma_gather's elem
    minimum, pair index fits int16 (no lo/hi table split).
  - Gather: 16 units/round, round-robin over 4 SWDGE queues. Queues 1-3
    dispatch asynchronously (~100ns) onto their own GpSimd Q7 core pairs;
    queue 0 blocks the engine and closes each wave => ~4x parallel
    descriptor generation (the baseline bottleneck).
  - Scatter: per dst tile, 0/1 selector matrices built on DVE (is_equal vs
    iota, bf16) contract edge chunks on TensorE into PSUM.
  - Epilogue: (psum + self term) scaled per-node, +0.1*u0 / +r*b3^T, bf16
    shard written to DRAM; per-group AllGather pieces rebuild the replicated
    table overlapped with compute.
"""

import math

import numpy as np

# ---------------- problem constants (hardcoded; kernel.py is standalone) ----
N_NODES = 50000
F_IN = 256
F_MID = 128
F_OUT = 64
ALPHA = 0.1
K_ITERS = 10
N_CORES = 8
P = 128
N_ROUNDS = K_ITERS + 2
RPC = N_NODES // N_CORES  # 6250
TILES = math.ceil(RPC / P)  # 49
GROUP_TILES = [6, 6, 6, 6, 6, 6, 6, 7]  # 8 groups over 49 tiles
QPAT = [0, 1, 2, 3]  # gather queue rotation; q0 first so waves close async


def _bf16(a):
    import ml_dtypes

    return np.asarray(a, dtype=np.float32).astype(ml_dtypes.bfloat16)


# ---------------------------------------------------------------- host prep
def preprocess(edge_index):
    """Graph-structure preprocessing (indices/degrees only)."""
    n = N_NODES
    src = np.asarray(edge_index[0], dtype=np.int64)
    dst = np.asarray(edge_index[1], dtype=np.int64)

    deg = np.bincount(dst, minlength=n).astype(np.float64) + 1.0  # + self loop
    dinv = 1.0 / np.sqrt(deg)
    rvec = dinv * np.bincount(dst, weights=dinv[src], minlength=n) + dinv * dinv

    core_of = dst // RPC
    local = dst - core_of * RPC
    tile_of = np.minimum(local // P, TILES - 1)
    # Table rows are laid out (group, core, local-row) so that per-group
    # AllGather pieces write contiguous ranges. Remap src node -> table row.
    gstart = np.concatenate([[0], np.cumsum(GROUP_TILES)])
    grow0 = gstart * P  # local row offset of each group (last entry clipped)
    rows_g = np.minimum(gstart[1:] * P, RPC) - grow0[:-1]
    base8 = np.concatenate([[0], np.cumsum(8 * rows_g)])
    group_of_tile = np.repeat(np.arange(len(GROUP_TILES)), GROUP_TILES)

    s_core = src // RPC
    s_local = src - s_core * RPC
    s_tile = np.minimum(s_local // P, TILES - 1)
    s_grp = group_of_tile[s_tile]
    rowpos = base8[s_grp] + s_core * rows_g[s_grp] + (s_local - grow0[s_grp])
    parity = rowpos & 1
    pidx = rowpos >> 1

    # chunk counts per (tile, parity), maxed over cores -> shared structure
    key = (core_of * TILES + tile_of) * 2 + parity
    cnt = np.bincount(key, minlength=N_CORES * TILES * 2).reshape(
        N_CORES, TILES, 2
    )
    CP = [
        [int(math.ceil(cnt[:, t, p].max() / P)) for p in (0, 1)]
        for t in range(TILES)
    ]
    ct = [CP[t][0] + CP[t][1] for t in range(TILES)]
    c0t = np.concatenate([[0], np.cumsum(ct)])  # global chunk start per tile
    c_tot = int(c0t[-1])
    chunk_parity = []
    for t in range(TILES):
        chunk_parity += [0] * CP[t][0] + [1] * CP[t][1]

    # groups / gather units
    assert gstart[-1] == TILES
    groups = []  # (tile0, ntiles, chunk0, nchunks)
    units = []  # (chunk0, nchunks, unit_of_group)
    for g in range(len(GROUP_TILES)):
        t0, t1 = int(gstart[g]), int(gstart[g + 1])
        groups.append((t0, t1 - t0, int(c0t[t0]), int(c0t[t1] - c0t[t0])))
        tm = t0 + (t1 - t0 + 1) // 2
        units.append((int(c0t[t0]), int(c0t[tm] - c0t[t0]), g))
        units.append((int(c0t[tm]), int(c0t[t1] - c0t[tm]), g))

    # sort edges by (core, tile, parity)
    order = np.argsort(key, kind="stable")
    pidx_s, local_s, key_s = pidx[order], local[order], key[order]
    bounds = np.searchsorted(key_s, np.arange(N_CORES * TILES * 2 + 1))

    per_core = []
    for c in range(N_CORES):
        idx_flat = np.zeros(c_tot * P, dtype=np.int16)
        dstl_flat = np.full(c_tot * P, -1.0, dtype=np.float32)
        pos = 0
        for t in range(TILES):
            for p in (0, 1):
                k = (c * TILES + t) * 2 + p
                a, b = bounds[k], bounds[k + 1]
                m = b - a
                cpad = CP[t][p] * P
                assert m <= cpad
                idx_flat[pos : pos + m] = pidx_s[a:b].astype(np.int16)
                dstl_flat[pos : pos + m] = (local_s[a:b] - t * P).astype(
                    np.float32
                )
                pos += cpad
        assert pos == c_tot * P

        idx_w = idx_flat.reshape(-1, 16).T.copy()  # [16, c_tot*8]
        idx_rep = np.tile(idx_w, (8, 1))  # [128, c_tot*8]
        dstl = dstl_flat.reshape(-1, P).T.copy()  # [128, c_tot]

        nodes = c * RPC + np.arange(TILES * P)
        valid = nodes < (c + 1) * RPC
        nodes_c = np.where(valid, nodes, c * RPC)
        col = lambda v: (
            np.where(valid, v[nodes_c], 0.0).reshape(TILES, P).T.astype(np.float32)
        ).copy()
        per_core.append(
            dict(
                idx=idx_rep,
                dstl=_bf16(dstl),
                cu0=col(1.0 / deg),
                cu0a=col(ALPHA / deg),
                cmid=col((1.0 - ALPHA) / deg),
                cfin=col(dinv),
                rcol=col(rvec),
            )
        )

    meta = dict(
        CP=CP,
        ct=ct,
        c0t=c0t,
        c_tot=c_tot,
        chunk_parity=chunk_parity,
        groups=groups,
        units=units,
        base8=[int(v) for v in base8],
    )
    return meta, per_core, dinv


def host_inputs(x, W1, b1, W3, b3, meta, per_core):
    W13 = (np.asarray(W1, np.float64) @ np.asarray(W3, np.float64)).astype(
        np.float32
    )
    b13 = (np.asarray(b1, np.float64) @ np.asarray(W3, np.float64)).astype(
        np.float32
    )
    b3 = np.asarray(b3, np.float32)
    iota = np.broadcast_to(np.arange(P, dtype=np.float32), (P, P)).copy()
    in_maps = []
    for c, pc in enumerate(per_core):
        xT = np.ascontiguousarray(np.asarray(x, np.float32)[c * RPC : (c + 1) * RPC].T)
        rb3 = (
            (pc["rcol"].reshape(P, TILES, 1) * b3.reshape(1, 1, F_OUT))
            .reshape(P, TILES * F_OUT)
            .astype(np.float32)
        )
        in_maps.append(
            dict(
                xT=_bf16(xT),
                W13=_bf16(W13),
                b13=b13.reshape(F_OUT, 1).copy(),
                rb3=rb3,
                idx=pc["idx"],
                dstl=pc["dstl"],
                cu0=pc["cu0"],
                cu0a=pc["cu0a"],
                cmid=pc["cmid"],
                cfin=pc["cfin"],
                iota=_bf16(iota),
            )
        )
    return in_maps


# ---------------------------------------------------------------- bass build
def build(meta):
    from concourse import bacc, mybir, tile
    from concourse.bass import AP
    from concourse import library_config
    from concourse.masks import make_identity

    f32 = mybir.dt.float32
    bf16 = mybir.dt.bfloat16
    i16 = mybir.dt.int16
    Copy = mybir.ActivationFunctionType.Copy
    Ident = mybir.ActivationFunctionType.Identity

    CP = meta["CP"]
    ct = meta["ct"]
    c0t = meta["c0t"]
    c_tot = meta["c_tot"]
    chunk_parity = meta["chunk_parity"]
    groups = meta["groups"]
    units = meta["units"]
    base8 = meta["base8"]
    n = N_NODES
    fo = F_OUT
    ctmax = max(ct)
    gcmax = max(g[3] for g in groups)
    rg = [list(range(N_CORES))]

    nc = bacc.Bacc(None, target_bir_lowering=False, debug=False, num_swdge_queues=4)

    xT_p = nc.declare_dram_parameter("xT", [F_IN, RPC], bf16, isOutput=False)
    W13_p = nc.declare_dram_parameter("W13", [F_IN, fo], bf16, isOutput=False)
    b13_p = nc.declare_dram_parameter("b13", [fo, 1], f32, isOutput=False)
    rb3_p = nc.declare_dram_parameter("rb3", [P, TILES * fo], f32, isOutput=False)
    idx_p = nc.declare_dram_parameter("idx", [P, c_tot * 8], i16, isOutput=False)
    dstl_p = nc.declare_dram_parameter("dstl", [P, c_tot], bf16, isOutput=False)
    cu0_p = nc.declare_dram_parameter("cu0", [P, TILES], f32, isOutput=False)
    cu0a_p = nc.declare_dram_parameter("cu0a", [P, TILES], f32, isOutput=False)
    cmid_p = nc.declare_dram_parameter("cmid", [P, TILES], f32, isOutput=False)
    cfin_p = nc.declare_dram_parameter("cfin", [P, TILES], f32, isOutput=False)
    iota_p = nc.declare_dram_parameter("iota", [P, P], bf16, isOutput=False)
    out_p = nc.declare_dram_parameter("out", [RPC, fo], f32, isOutput=True)

    # node tables (bf16, viewed as [n/2, 128] pair rows for the gather) and
    # per-round local shards
    T = [
        nc.dram_tensor(f"T{k}", [n, fo], bf16, addr_space="Shared")
        for k in range(N_ROUNDS)
    ]
    shard = [nc.dram_tensor(f"sh{k}", [RPC, fo], bf16) for k in range(N_ROUNDS)]

    qi = 0  # gather queue rotation counter

    with tile.TileContext(nc) as tc:
        nc.gpsimd.load_library(library_config.mlp)
        with (
            tc.tile_pool(name="const", bufs=1) as cp,
            tc.tile_pool(name="psA", bufs=2, space="PSUM") as psA,
            tc.tile_pool(name="psT", bufs=2, space="PSUM") as psT,
            tc.tile_pool(name="dense", bufs=3) as dp,
            tc.tile_pool(name="gat", bufs=3) as gp,
            tc.tile_pool(name="sel", bufs=4) as sp,
            tc.tile_pool(name="outt", bufs=4) as op,
            tc.tile_pool(name="tmpp", bufs=4) as tp_,
            tc.tile_pool(name="ps2", bufs=4, space="PSUM") as pp2,
        ):
            # ---------------- resident constants ----------------
            idx_sb = cp.tile([P, c_tot * 8], i16)
            nc.sync.dma_start(idx_sb[:], idx_p[:])
            dstl_sb = cp.tile([P, c_tot], bf16)
            nc.sync.dma_start(dstl_sb[:], dstl_p[:])
            cu0_sb = cp.tile([P, TILES], f32)
            nc.sync.dma_start(cu0_sb[:], cu0_p[:])
            cu0a_sb = cp.tile([P, TILES], f32)
            nc.sync.dma_start(cu0a_sb[:], cu0a_p[:])
            cmid_sb = cp.tile([P, TILES], f32)
            nc.sync.dma_start(cmid_sb[:], cmid_p[:])
            cfin_sb = cp.tile([P, TILES], f32)
            nc.sync.dma_start(cfin_sb[:], cfin_p[:])
            iota_sb = cp.tile([P, P], bf16)
            nc.sync.dma_start(iota_sb[:], iota_p[:])
            rb3_sb = cp.tile([P, TILES * fo], f32)
            nc.sync.dma_start(rb3_sb[:], rb3_p[:])
            b13_sb = cp.tile([fo, 1], f32)
            nc.sync.dma_start(b13_sb[:], b13_p[:])
            w13_sb = cp.tile([P, (F_IN // P) * fo], bf16)
            for kk in range(F_IN // P):
                nc.sync.dma_start(
                    w13_sb[:, kk * fo : (kk + 1) * fo],
                    W13_p[kk * P : (kk + 1) * P, :],
                )
            ident = cp.tile([P, P], f32)
            make_identity(nc, ident[:])
            # node-major local state, one 64-col slot per tile
            z_sb = cp.tile([P, TILES * fo], bf16)
            uprev_sb = cp.tile([P, TILES * fo], bf16)
            u0s_sb = cp.tile([P, TILES * fo], bf16)

            # ---------------- dense phase: z = dinv*(x@W13 + b13) ----------
            NBLK = 512
            nblocks = math.ceil(RPC / NBLK)
            for bi in range(nblocks):
                w = min(NBLK, RPC - bi * NBLK)
                ps = psA.tile([fo, NBLK], f32, tag="ps")
                for kk in range(F_IN // P):
                    xt = dp.tile([P, NBLK], bf16, tag="xt")
                    nc.sync.dma_start(
                        xt[:, :w],
                        xT_p[kk * P : (kk + 1) * P, bi * NBLK : bi * NBLK + w],
                    )
                    nc.tensor.matmul(
                        ps[:, :w],
                        lhsT=w13_sb[:, kk * fo : (kk + 1) * fo],
                        rhs=xt[:, :w],
                        start=(kk == 0),
                        stop=(kk == F_IN // P - 1),
                    )
                zt = dp.tile([fo, NBLK], f32, tag="zt")
                nc.scalar.activation(zt[:, :w], ps[:, :w], Ident, bias=b13_sb[:, :1])
                for s in range(math.ceil(w / P)):
                    sw = min(P, w - s * P)
                    t = (bi * NBLK + s * P) // P  # global tile index
                    pt = psT.tile([P, fo], f32, tag="pt")
                    nc.tensor.transpose(
                        pt[:sw, :], zt[:, s * P : s * P + sw], ident[:fo, :fo]
                    )
                    nc.scalar.activation(
                        z_sb[:sw, t * fo : t * fo + fo],
                        pt[:sw, :],
                        Copy,
                        scale=cfin_sb[:sw, t : t + 1],
                    )
                    nc.sync.dma_start(
                        shard[0][t * P : t * P + sw, :],
                        z_sb[:sw, t * fo : t * fo + fo],
                    )
            for g, (t0, ntiles, _, _) in enumerate(groups):
                r0 = t0 * P
                r1 = min((t0 + ntiles) * P, RPC)
                nc.gpsimd.collective_compute(
                    "AllGather",
                    mybir.AluOpType.bypass,
                    replica_groups=rg,
                    ins=[shard[0][r0:r1, :]],
                    outs=[T[0][base8[g] : base8[g] + 8 * (r1 - r0), :]],
                )

            # ---------------- propagation rounds ----------------
            for rnd in range(N_ROUNDS):
                Tin = T[rnd][:, :].rearrange("(a b) e -> a (b e)", b=2)
                for g, (t0, ntiles, gc0, gnc) in enumerate(groups):
                    gb = gp.tile([P, gcmax * P], bf16, tag="gb")
                    for u in range(2):
                        uc0, unc, _ = units[g * 2 + u]
                        nc.gpsimd.dma_gather(
                            out_ap=gb[
                                :, (uc0 - gc0) * P : (uc0 - gc0 + unc) * P
                            ].rearrange("p (c e) -> p c e", e=P),
                            in_ap=Tin,
                            idxs_ap=idx_sb[:, uc0 * 8 : (uc0 + unc) * 8],
                            num_idxs=unc * P,
                            num_idxs_reg=unc * P,
                            elem_size=P,
                            single_packet=False,
                            queue_num=QPAT[qi % 4],
                        )
                        qi += 1
                    for tl in range(ntiles):
                        t = t0 + tl
                        rows = min(P, RPC - t * P)
                        tc0 = int(c0t[t])
                        tct = ct[t]
                        # selector build: S[p, c, r] = (dstl[p, c] == r)
                        st = sp.tile([P, ctmax * P], bf16, tag="st")
                        din = dstl_sb[:, tc0 : tc0 + tct].to_broadcast(
                            [P, tct, P]
                        )
                        iin = AP(
                            iota_sb[:].tensor,
                            iota_sb[:].offset,
                            [iota_sb[:].ap[0], [0, tct], [1, P]],
                        )
                        sout = st[:, 0 : tct * P].rearrange(
                            "p (c r) -> p c r", r=P
                        )
                        nc.vector.tensor_tensor(
                            out=sout, in0=din, in1=iin, op=mybir.AluOpType.is_equal
                        )
                        pt = pp2.tile([P, fo], f32, tag="pt2")
                        for j in range(tct):
                            gcol = (tc0 - gc0 + j) * P + chunk_parity[tc0 + j] * fo
                            nc.tensor.matmul(
                                pt[:, :],
                                lhsT=st[:, j * P : (j + 1) * P],
                                rhs=gb[:, gcol : gcol + fo],
                                start=(j == 0),
                                stop=(j == tct - 1),
                            )
                        # epilogue: self-loop add + per-node scaling
                        so = t * fo
                        tmp = tp_.tile([P, fo], f32, tag="tmp")
                        if rnd == 0:
                            nc.vector.tensor_add(
                                tmp[:rows, :], pt[:rows, :], z_sb[:rows, so : so + fo]
                            )
                            nc.scalar.activation(
                                uprev_sb[:rows, so : so + fo],
                                tmp[:rows, :],
                                Copy,
                                scale=cu0_sb[:rows, t : t + 1],
                            )
                            nc.scalar.activation(
                                u0s_sb[:rows, so : so + fo],
                                tmp[:rows, :],
                                Copy,
                                scale=cu0a_sb[:rows, t : t + 1],
                            )
                            nc.sync.dma_start(
                                shard[1][t * P : t * P + rows, :],
                                uprev_sb[:rows, so : so + fo],
                            )
                        elif rnd < N_ROUNDS - 1:
                            nc.vector.tensor_add(
                                tmp[:rows, :],
                                pt[:rows, :],
                                uprev_sb[:rows, so : so + fo],
                            )
                            tmp2 = tp_.tile([P, fo], f32, tag="tmp2")
                            nc.scalar.activation(
                                tmp2[:rows, :],
                                tmp[:rows, :],
                                Copy,
                                scale=cmid_sb[:rows, t : t + 1],
                            )
                            nc.vector.tensor_add(
                                uprev_sb[:rows, so : so + fo],
                                tmp2[:rows, :],
                                u0s_sb[:rows, so : so + fo],
                            )
                            nc.sync.dma_start(
                                shard[rnd + 1][t * P : t * P + rows, :],
                                uprev_sb[:rows, so : so + fo],
                            )
                        else:
                            nc.vector.tensor_add(
                                tmp[:rows, :],
                                pt[:rows, :],
                                uprev_sb[:rows, so : so + fo],
                            )
                            tmp2 = tp_.tile([P, fo], f32, tag="tmp2")
                            nc.scalar.activation(
                                tmp2[:rows, :],
                                tmp[:rows, :],
                                Copy,
                                scale=cfin_sb[:rows, t : t + 1],
                            )
                            ot = op.tile([P, fo], f32, tag="ot")
                            nc.vector.tensor_add(
                                ot[:rows, :],
                                tmp2[:rows, :],
                                rb3_sb[:rows, so : so + fo],
                            )
                            nc.sync.dma_start(
                                out_p[t * P : t * P + rows, :], ot[:rows, :]
                            )
                    # per-group AllGather piece into next round's table
                    if rnd < N_ROUNDS - 1:
                        r0 = t0 * P
                        r1 = min((t0 + ntiles) * P, RPC)
                        nc.gpsimd.collective_compute(
                            "AllGather",
                            mybir.AluOpType.bypass,
                            replica_groups=rg,
                            ins=[shard[rnd + 1][r0:r1, :]],
                            outs=[
                                T[rnd + 1][base8[g] : base8[g] + 8 * (r1 - r0), :]
                            ],
                        )
    nc.compile()
    return nc


# ---------------------------------------------------------------- runner
def run(x, edge_index, W1, b1, W3, b3, trace=False):
    from concourse.bass_utils import run_bass_kernel_spmd

    meta, per_core, _ = preprocess(edge_index)
    in_maps = host_inputs(x, W1, b1, W3, b3, meta, per_core)
    nc = build(meta)
    res = run_bass_kernel_spmd(
        nc, in_maps, core_ids=list(range(N_CORES)), trace=trace
    )
    out = np.concatenate(
        [res.results[i]["out"] for i in range(N_CORES)], axis=0
    )
    return out.astype(np.float32), res


def kernel(**inputs):
    out, _ = run(
        inputs["x"],
        inputs["edge_index"],
        inputs["W1"],
        inputs["b1"],
        inputs["W3"],
        inputs["b3"],
    )
    return out


# revision 13
# speedup vs baseline: 2.2780x; 1.1391x over previous
"""APPNP GNN (GCN -> 10x APPNP -> GCN) on 8 TRN2 NeuronCores.

Math refactoring (exact, linear algebra):
  Reference: P = D^-1/2 (A+I) D^-1/2  (on 800k random edges + self loops)
     h = P(x@W1+b1);  h_{k+1} = 0.9*P*h_k + 0.1*h_0 (10 iters);  out = P(h@W3+b3)
  Right-multiplication commutes with P, so fold W3 in early (W13 = W1@W3):
     g_0 = P(x@W13 + b13);  g_{k+1} = 0.9*P*g_k + 0.1*g_0;  out = P*g_K + r*b3^T
  with b13 = b1@W3, r = P@1.  Feature dim drops 128 -> 64 for all propagations.
  Substituting u_k = D^-1/2 g_k makes the inner op a plain adjacency sum:
     z   = D^-1/2 (x@W13 + b13)
     u_0 = D^-1 B z                  (B = A+I, unweighted 0/1)
     u_{k+1} = 0.9 D^-1 B u_k + 0.1 u_0
     out = D^-1/2 B u_10 + r b3^T
  The self-loop (+I) term is applied as a local tile add (u_prev kept in
  SBUF), so only the 800k real edges go through the gather path.

Device strategy (per core, dst rows sharded 8 ways, 12 rounds of y = B u):
  - Node table in DRAM as bf16 [25000 pairs, 128]: node n's 64 features at
    row n>>1, halves selected by n&1. 256B rows satisfy dma_gather's elem
    minimum, pair index fits int16 (no lo/hi table split).
  - Gather: 16 units/round, round-robin over 4 SWDGE queues. Queues 1-3
    dispatch asynchronously (~100ns) onto their own GpSimd Q7 core pairs;
    queue 0 blocks the engine and closes each wave => ~4x parallel
    descriptor generation (the baseline bottleneck).
  - Scatter: per dst tile, 0/1 selector matrices built on DVE (is_equal vs
    iota, bf16) contract edge chunks on TensorE into PSUM.
  - Epilogue: (psum + self term) scaled per-node, +0.1*u0 / +r*b3^T, bf16
    shard written to DRAM; per-group AllGather pieces rebuild the replicated
    table overlapped with compute.
"""

import math

import numpy as np

# ---------------- problem constants (hardcoded; kernel.py is standalone) ----
N_NODES = 50000
F_IN = 256
F_MID = 128
F_OUT = 64
ALPHA = 0.1
K_ITERS = 10
N_CORES = 8
P = 128
N_ROUNDS = K_ITERS + 2
RPC = N_NODES // N_CORES  # 6250
TILES = math.ceil(RPC / P)  # 49
GROUP_TILES = [7, 7, 7, 7, 7, 7, 6, 1]  # 8 groups over 49 tiles; small tail
CC_LAG = 2  # collective pieces trail the gather stream by this many groups


def _bf16(a):
    import ml_dtypes

    return np.asarray(a, dtype=np.float32).astype(ml_dtypes.bfloat16)


# ---------------------------------------------------------------- host prep
def preprocess(edge_index):
    """Graph-structure preprocessing (indices/degrees only)."""
    n = N_NODES
    src = np.asarray(edge_index[0], dtype=np.int64)
    dst = np.asarray(edge_index[1], dtype=np.int64)

    deg = np.bincount(dst, minlength=n).astype(np.float64) + 1.0  # + self loop
    dinv = 1.0 / np.sqrt(deg)
    rvec = dinv * np.bincount(dst, weights=dinv[src], minlength=n) + dinv * dinv

    core_of = dst // RPC
    local = dst - core_of * RPC
    tile_of = np.minimum(local // P, TILES - 1)
    # Table rows are laid out (group, core, local-row) so that per-group
    # AllGather pieces write contiguous ranges. Remap src node -> table row.
    gstart = np.concatenate([[0], np.cumsum(GROUP_TILES)])
    grow0 = gstart * P  # local row offset of each group (last entry clipped)
    rows_g = np.minimum(gstart[1:] * P, RPC) - grow0[:-1]
    base8 = np.concatenate([[0], np.cumsum(8 * rows_g)])
    group_of_tile = np.repeat(np.arange(len(GROUP_TILES)), GROUP_TILES)

    s_core = src // RPC
    s_local = src - s_core * RPC
    s_tile = np.minimum(s_local // P, TILES - 1)
    s_grp = group_of_tile[s_tile]
    rowpos = base8[s_grp] + s_core * rows_g[s_grp] + (s_local - grow0[s_grp])
    parity = rowpos & 1
    pidx = rowpos >> 1

    # chunk counts per (tile, parity), maxed over cores -> shared structure
    key = (core_of * TILES + tile_of) * 2 + parity
    cnt = np.bincount(key, minlength=N_CORES * TILES * 2).reshape(
        N_CORES, TILES, 2
    )
    CP = [
        [int(math.ceil(cnt[:, t, p].max() / P)) for p in (0, 1)]
        for t in range(TILES)
    ]
    ct = [CP[t][0] + CP[t][1] for t in range(TILES)]
    c0t = np.concatenate([[0], np.cumsum(ct)])  # global chunk start per tile
    c_tot = int(c0t[-1])
    chunk_parity = []
    for t in range(TILES):
        chunk_parity += [0] * CP[t][0] + [1] * CP[t][1]

    # groups / gather units
    assert gstart[-1] == TILES
    groups = []  # (tile0, ntiles, chunk0, nchunks)
    units = []  # (chunk0, nchunks, unit_of_group)
    for g in range(len(GROUP_TILES)):
        t0, t1 = int(gstart[g]), int(gstart[g + 1])
        groups.append((t0, t1 - t0, int(c0t[t0]), int(c0t[t1] - c0t[t0])))
        tm = t0 + (t1 - t0 + 1) // 2
        units.append((int(c0t[t0]), int(c0t[tm] - c0t[t0]), g))
        units.append((int(c0t[tm]), int(c0t[t1] - c0t[tm]), g))

    # sort edges by (core, tile, parity)
    order = np.argsort(key, kind="stable")
    pidx_s, local_s, key_s = pidx[order], local[order], key[order]
    bounds = np.searchsorted(key_s, np.arange(N_CORES * TILES * 2 + 1))

    per_core = []
    for c in range(N_CORES):
        idx_flat = np.zeros(c_tot * P, dtype=np.int16)
        dstl_flat = np.full(c_tot * P, -1.0, dtype=np.float32)
        pos = 0
        for t in range(TILES):
            for p in (0, 1):
                k = (c * TILES + t) * 2 + p
                a, b = bounds[k], bounds[k + 1]
                m = b - a
                cpad = CP[t][p] * P
                assert m <= cpad
                idx_flat[pos : pos + m] = pidx_s[a:b].astype(np.int16)
                dstl_flat[pos : pos + m] = (local_s[a:b] - t * P).astype(
                    np.float32
                )
                pos += cpad
        assert pos == c_tot * P

        idx_w = idx_flat.reshape(-1, 16).T.copy()  # [16, c_tot*8]
        idx_rep = np.tile(idx_w, (8, 1))  # [128, c_tot*8]
        dstl = dstl_flat.reshape(-1, P).T.copy()  # [128, c_tot]

        nodes = c * RPC + np.arange(TILES * P)
        valid = nodes < (c + 1) * RPC
        nodes_c = np.where(valid, nodes, c * RPC)
        col = lambda v: (
            np.where(valid, v[nodes_c], 0.0).reshape(TILES, P).T.astype(np.float32)
        ).copy()
        per_core.append(
            dict(
                idx=idx_rep,
                dstl=_bf16(dstl),
                cu0=col(1.0 / deg),
                cu0a=col(ALPHA / deg),
                cmid=col((1.0 - ALPHA) / deg),
                cfin=col(dinv),
                rcol=col(rvec),
            )
        )

    meta = dict(
        CP=CP,
        ct=ct,
        c0t=c0t,
        c_tot=c_tot,
        chunk_parity=chunk_parity,
        groups=groups,
        units=units,
        base8=[int(v) for v in base8],
    )
    return meta, per_core, dinv


def host_inputs(x, W1, b1, W3, b3, meta, per_core):
    W13 = (np.asarray(W1, np.float64) @ np.asarray(W3, np.float64)).astype(
        np.float32
    )
    b13 = (np.asarray(b1, np.float64) @ np.asarray(W3, np.float64)).astype(
        np.float32
    )
    b3 = np.asarray(b3, np.float32)
    iota = np.broadcast_to(np.arange(P, dtype=np.float32), (P, P)).copy()
    in_maps = []
    for c, pc in enumerate(per_core):
        xT = np.ascontiguousarray(np.asarray(x, np.float32)[c * RPC : (c + 1) * RPC].T)
        rb3 = (
            (pc["rcol"].reshape(P, TILES, 1) * b3.reshape(1, 1, F_OUT))
            .reshape(P, TILES * F_OUT)
            .astype(np.float32)
        )
        in_maps.append(
            dict(
                xT=_bf16(xT),
                W13=_bf16(W13),
                b13=b13.reshape(F_OUT, 1).copy(),
                rb3=rb3,
                idx=pc["idx"],
                dstl=pc["dstl"],
                cu0=pc["cu0"],
                cu0a=pc["cu0a"],
                cmid=pc["cmid"],
                cfin=pc["cfin"],
                iota=_bf16(iota),
            )
        )
    return in_maps


# ---------------------------------------------------------------- bass build
def build(meta):
    from concourse import bacc, mybir, tile
    from concourse.bass import AP
    from concourse import library_config
    from concourse.masks import make_identity

    f32 = mybir.dt.float32
    bf16 = mybir.dt.bfloat16
    i16 = mybir.dt.int16
    Copy = mybir.ActivationFunctionType.Copy
    Ident = mybir.ActivationFunctionType.Identity

    CP = meta["CP"]
    ct = meta["ct"]
    c0t = meta["c0t"]
    c_tot = meta["c_tot"]
    chunk_parity = meta["chunk_parity"]
    groups = meta["groups"]
    units = meta["units"]
    base8 = meta["base8"]
    n = N_NODES
    fo = F_OUT
    ctmax = max(ct)
    gcmax = max(g[3] for g in groups)
    rg = [list(range(N_CORES))]

    nc = bacc.Bacc(None, target_bir_lowering=False, debug=False, num_swdge_queues=4)

    xT_p = nc.declare_dram_parameter("xT", [F_IN, RPC], bf16, isOutput=False)
    W13_p = nc.declare_dram_parameter("W13", [F_IN, fo], bf16, isOutput=False)
    b13_p = nc.declare_dram_parameter("b13", [fo, 1], f32, isOutput=False)
    rb3_p = nc.declare_dram_parameter("rb3", [P, TILES * fo], f32, isOutput=False)
    idx_p = nc.declare_dram_parameter("idx", [P, c_tot * 8], i16, isOutput=False)
    dstl_p = nc.declare_dram_parameter("dstl", [P, c_tot], bf16, isOutput=False)
    cu0_p = nc.declare_dram_parameter("cu0", [P, TILES], f32, isOutput=False)
    cu0a_p = nc.declare_dram_parameter("cu0a", [P, TILES], f32, isOutput=False)
    cmid_p = nc.declare_dram_parameter("cmid", [P, TILES], f32, isOutput=False)
    cfin_p = nc.declare_dram_parameter("cfin", [P, TILES], f32, isOutput=False)
    iota_p = nc.declare_dram_parameter("iota", [P, P], bf16, isOutput=False)
    out_p = nc.declare_dram_parameter("out", [RPC, fo], f32, isOutput=True)

    # node tables (bf16, viewed as [n/2, 128] pair rows for the gather) and
    # per-round local shards
    T = [
        nc.dram_tensor(f"T{k}", [n, fo], bf16, addr_space="Shared")
        for k in range(N_ROUNDS)
    ]
    shard = [nc.dram_tensor(f"sh{k}", [RPC, fo], bf16) for k in range(N_ROUNDS)]

    qi = 0  # gather queue rotation counter

    with tile.TileContext(nc) as tc:
        nc.gpsimd.load_library(library_config.mlp)
        with (
            tc.tile_pool(name="const", bufs=1) as cp,
            tc.tile_pool(name="psA", bufs=2, space="PSUM") as psA,
            tc.tile_pool(name="psT", bufs=2, space="PSUM") as psT,
            tc.tile_pool(name="dense", bufs=3) as dp,
            tc.tile_pool(name="gat", bufs=4) as gp,
            tc.tile_pool(name="sel", bufs=4) as sp,
            tc.tile_pool(name="outt", bufs=4) as op,
            tc.tile_pool(name="tmpp", bufs=4) as tp_,
            tc.tile_pool(name="ps2", bufs=4, space="PSUM") as pp2,
        ):
            # ---------------- resident constants ----------------
            idx_sb = cp.tile([P, c_tot * 8], i16)
            nc.sync.dma_start(idx_sb[:], idx_p[:])
            dstl_sb = cp.tile([P, c_tot], bf16)
            nc.sync.dma_start(dstl_sb[:], dstl_p[:])
            cu0_sb = cp.tile([P, TILES], f32)
            nc.sync.dma_start(cu0_sb[:], cu0_p[:])
            cu0a_sb = cp.tile([P, TILES], f32)
            nc.sync.dma_start(cu0a_sb[:], cu0a_p[:])
            cmid_sb = cp.tile([P, TILES], f32)
            nc.sync.dma_start(cmid_sb[:], cmid_p[:])
            cfin_sb = cp.tile([P, TILES], f32)
            nc.sync.dma_start(cfin_sb[:], cfin_p[:])
            iota_sb = cp.tile([P, P], bf16)
            nc.sync.dma_start(iota_sb[:], iota_p[:])
            rb3_sb = cp.tile([P, TILES * fo], f32)
            nc.sync.dma_start(rb3_sb[:], rb3_p[:])
            b13_sb = cp.tile([fo, 1], f32)
            nc.sync.dma_start(b13_sb[:], b13_p[:])
            w13_sb = cp.tile([P, (F_IN // P) * fo], bf16)
            for kk in range(F_IN // P):
                nc.sync.dma_start(
                    w13_sb[:, kk * fo : (kk + 1) * fo],
                    W13_p[kk * P : (kk + 1) * P, :],
                )
            ident = cp.tile([P, P], f32)
            make_identity(nc, ident[:])
            # node-major local state, one 64-col slot per tile
            z_sb = cp.tile([P, TILES * fo], bf16)
            uprev_sb = cp.tile([P, TILES * fo], bf16)
            u0s_sb = cp.tile([P, TILES * fo], bf16)

            # ---------------- dense phase: z = dinv*(x@W13 + b13) ----------
            NBLK = 512
            nblocks = math.ceil(RPC / NBLK)
            for bi in range(nblocks):
                w = min(NBLK, RPC - bi * NBLK)
                ps = psA.tile([fo, NBLK], f32, tag="ps")
                for kk in range(F_IN // P):
                    xt = dp.tile([P, NBLK], bf16, tag="xt")
                    nc.sync.dma_start(
                        xt[:, :w],
                        xT_p[kk * P : (kk + 1) * P, bi * NBLK : bi * NBLK + w],
                    )
                    nc.tensor.matmul(
                        ps[:, :w],
                        lhsT=w13_sb[:, kk * fo : (kk + 1) * fo],
                        rhs=xt[:, :w],
                        start=(kk == 0),
                        stop=(kk == F_IN // P - 1),
                    )
                zt = dp.tile([fo, NBLK], f32, tag="zt")
                nc.scalar.activation(zt[:, :w], ps[:, :w], Ident, bias=b13_sb[:, :1])
                for s in range(math.ceil(w / P)):
                    sw = min(P, w - s * P)
                    t = (bi * NBLK + s * P) // P  # global tile index
                    pt = psT.tile([P, fo], f32, tag="pt")
                    nc.tensor.transpose(
                        pt[:sw, :], zt[:, s * P : s * P + sw], ident[:fo, :fo]
                    )
                    nc.scalar.activation(
                        z_sb[:sw, t * fo : t * fo + fo],
                        pt[:sw, :],
                        Copy,
                        scale=cfin_sb[:sw, t : t + 1],
                    )
                    nc.sync.dma_start(
                        shard[0][t * P : t * P + sw, :],
                        z_sb[:sw, t * fo : t * fo + fo],
                    )
            for g, (t0, ntiles, _, _) in enumerate(groups):
                r0 = t0 * P
                r1 = min((t0 + ntiles) * P, RPC)
                nc.gpsimd.collective_compute(
                    "AllGather",
                    mybir.AluOpType.bypass,
                    replica_groups=rg,
                    ins=[shard[0][r0:r1, :]],
                    outs=[T[0][base8[g] : base8[g] + 8 * (r1 - r0), :]],
                )

            # ---------------- propagation rounds ----------------
            # greedy per-round queue assignment balancing generation load
            def round_queues():
                loads = [0.0, 0.0, 0.0, 0.0]
                qs = []
                nunits = len(units)
                for ui, (_, unc, _) in enumerate(units):
                    avoid0 = ui >= nunits - 3  # keep round tail async
                    cand = range(1, 4) if avoid0 else range(4)
                    q = min(cand, key=lambda c: loads[c])
                    loads[q] += unc
                    qs.append(q)
                return qs

            qsched = round_queues()

            def emit_cc(rnd, g):
                t0, ntiles, _, _ = groups[g]
                r0 = t0 * P
                r1 = min((t0 + ntiles) * P, RPC)
                nc.gpsimd.collective_compute(
                    "AllGather",
                    mybir.AluOpType.bypass,
                    replica_groups=rg,
                    ins=[shard[rnd + 1][r0:r1, :]],
                    outs=[T[rnd + 1][base8[g] : base8[g] + 8 * (r1 - r0), :]],
                )

            for rnd in range(N_ROUNDS):
                Tin = T[rnd][:, :].rearrange("(a b) e -> a (b e)", b=2)
                for g, (t0, ntiles, gc0, gnc) in enumerate(groups):
                    gb = gp.tile([P, gcmax * P], bf16, tag="gb")
                    for u in range(2):
                        uc0, unc, _ = units[g * 2 + u]
                        if unc == 0:
                            continue
                        nc.gpsimd.dma_gather(
                            out_ap=gb[
                                :, (uc0 - gc0) * P : (uc0 - gc0 + unc) * P
                            ].rearrange("p (c e) -> p c e", e=P),
                            in_ap=Tin,
                            idxs_ap=idx_sb[:, uc0 * 8 : (uc0 + unc) * 8],
                            num_idxs=unc * P,
                            num_idxs_reg=unc * P,
                            elem_size=P,
                            single_packet=False,
                            queue_num=qsched[g * 2 + u],
                        )
                    if rnd < N_ROUNDS - 1 and g - CC_LAG >= 0:
                        emit_cc(rnd, g - CC_LAG)
                    for tl in range(ntiles):
                        t = t0 + tl
                        rows = min(P, RPC - t * P)
                        tc0 = int(c0t[t])
                        tct = ct[t]
                        # selector build: S[p, c, r] = (dstl[p, c] == r)
                        st = sp.tile([P, ctmax * P], bf16, tag="st")
                        din = dstl_sb[:, tc0 : tc0 + tct].to_broadcast(
                            [P, tct, P]
                        )
                        iin = AP(
                            iota_sb[:].tensor,
                            iota_sb[:].offset,
                            [iota_sb[:].ap[0], [0, tct], [1, P]],
                        )
                        sout = st[:, 0 : tct * P].rearrange(
                            "p (c r) -> p c r", r=P
                        )
                        nc.vector.tensor_tensor(
                            out=sout, in0=din, in1=iin, op=mybir.AluOpType.is_equal
                        )
                        pt = pp2.tile([P, fo], f32, tag="pt2")
                        for j in range(tct):
                            gcol = (tc0 - gc0 + j) * P + chunk_parity[tc0 + j] * fo
                            nc.tensor.matmul(
                                pt[:, :],
                                lhsT=st[:, j * P : (j + 1) * P],
                                rhs=gb[:, gcol : gcol + fo],
                                start=(j == 0),
                                stop=(j == tct - 1),
                            )
                        # epilogue: self-loop add + per-node scaling
                        so = t * fo
                        tmp = tp_.tile([P, fo], f32, tag="tmp")
                        if rnd == 0:
                            nc.vector.tensor_add(
                                tmp[:rows, :], pt[:rows, :], z_sb[:rows, so : so + fo]
                            )
                            nc.scalar.activation(
                                uprev_sb[:rows, so : so + fo],
                                tmp[:rows, :],
                                Copy,
                                scale=cu0_sb[:rows, t : t + 1],
                            )
                            nc.scalar.activation(
                                u0s_sb[:rows, so : so + fo],
                                tmp[:rows, :],
                                Copy,
                                scale=cu0a_sb[:rows, t : t + 1],
                            )
                            nc.sync.dma_start(
                                shard[1][t * P : t * P + rows, :],
                                uprev_sb[:rows, so : so + fo],
                            )
                        elif rnd < N_ROUNDS - 1:
                            nc.vector.tensor_add(
                                tmp[:rows, :],
                                pt[:rows, :],
                                uprev_sb[:rows, so : so + fo],
                            )
                            tmp2 = tp_.tile([P, fo], f32, tag="tmp2")
                            nc.scalar.activation(
                                tmp2[:rows, :],
                                tmp[:rows, :],
                                Copy,
                                scale=cmid_sb[:rows, t : t + 1],
                            )
                            nc.vector.tensor_add(
                                uprev_sb[:rows, so : so + fo],
                                tmp2[:rows, :],
                                u0s_sb[:rows, so : so + fo],
                            )
                            nc.sync.dma_start(
                                shard[rnd + 1][t * P : t * P + rows, :],
                                uprev_sb[:rows, so : so + fo],
                            )
                        else:
                            nc.vector.tensor_add(
                                tmp[:rows, :],
                                pt[:rows, :],
                                uprev_sb[:rows, so : so + fo],
                            )
                            tmp2 = tp_.tile([P, fo], f32, tag="tmp2")
                            nc.scalar.activation(
                                tmp2[:rows, :],
                                tmp[:rows, :],
                                Copy,
                                scale=cfin_sb[:rows, t : t + 1],
                            )
                            ot = op.tile([P, fo], f32, tag="ot")
                            nc.vector.tensor_add(
                                ot[:rows, :],
                                tmp2[:rows, :],
                                rb3_sb[:rows, so : so + fo],
                            )
                            nc.sync.dma_start(
                                out_p[t * P : t * P + rows, :], ot[:rows, :]
                            )
                # trailing collective pieces for the last groups
                if rnd < N_ROUNDS - 1:
                    for g in range(len(groups) - CC_LAG, len(groups)):
                        emit_cc(rnd, g)
    nc.compile()
    return nc


# ---------------------------------------------------------------- runner
def run(x, edge_index, W1, b1, W3, b3, trace=False):
    from concourse.bass_utils import run_bass_kernel_spmd

    meta, per_core, _ = preprocess(edge_index)
    in_maps = host_inputs(x, W1, b1, W3, b3, meta, per_core)
    nc = build(meta)
    res = run_bass_kernel_spmd(
        nc, in_maps, core_ids=list(range(N_CORES)), trace=trace
    )
    out = np.concatenate(
        [res.results[i]["out"] for i in range(N_CORES)], axis=0
    )
    return out.astype(np.float32), res


def kernel(**inputs):
    out, _ = run(
        inputs["x"],
        inputs["edge_index"],
        inputs["W1"],
        inputs["b1"],
        inputs["W3"],
        inputs["b3"],
    )
    return out


# revision 15
# speedup vs baseline: 2.5157x; 1.1044x over previous
"""APPNP GNN (GCN -> 10x APPNP -> GCN) on 8 TRN2 NeuronCores.

Math refactoring (exact, linear algebra):
  Reference: P = D^-1/2 (A+I) D^-1/2  (on 800k random edges + self loops)
     h = P(x@W1+b1);  h_{k+1} = 0.9*P*h_k + 0.1*h_0 (10 iters);  out = P(h@W3+b3)
  Right-multiplication commutes with P, so fold W3 in early (W13 = W1@W3):
     g_0 = P(x@W13 + b13);  g_{k+1} = 0.9*P*g_k + 0.1*g_0;  out = P*g_K + r*b3^T
  with b13 = b1@W3, r = P@1.  Feature dim drops 128 -> 64 for all propagations.
  Substituting u_k = D^-1/2 g_k makes the inner op a plain adjacency sum:
     z   = D^-1/2 (x@W13 + b13)
     u_0 = D^-1 B z                  (B = A+I, unweighted 0/1)
     u_{k+1} = 0.9 D^-1 B u_k + 0.1 u_0
     out = D^-1/2 B u_10 + r b3^T
  The self-loop (+I) term is applied as a local tile add (u_prev kept in
  SBUF), so only the 800k real edges go through the gather path.

Device strategy (per core, dst rows sharded 8 ways, 12 rounds of y = B u):
  - Node table in DRAM as bf16 [25000 pairs, 128]: node n's 64 features at
    row n>>1, halves selected by n&1. 256B rows satisfy dma_gather's elem
    minimum, pair index fits int16 (no lo/hi table split).
  - Gather: 16 units/round, round-robin over 4 SWDGE queues. Queues 1-3
    dispatch asynchronously (~100ns) onto their own GpSimd Q7 core pairs;
    queue 0 blocks the engine and closes each wave => ~4x parallel
    descriptor generation (the baseline bottleneck).
  - Scatter: per dst tile, 0/1 selector matrices built on DVE (is_equal vs
    iota, bf16) contract edge chunks on TensorE into PSUM.
  - Epilogue: (psum + self term) scaled per-node, +0.1*u0 / +r*b3^T, bf16
    shard written to DRAM; per-group AllGather pieces rebuild the replicated
    table overlapped with compute.
"""

import math

import numpy as np

# ---------------- problem constants (hardcoded; kernel.py is standalone) ----
N_NODES = 50000
F_IN = 256
F_MID = 128
F_OUT = 64
ALPHA = 0.1
K_ITERS = 10
N_CORES = 8
P = 128
N_ROUNDS = K_ITERS + 2
RPC = N_NODES // N_CORES  # 6250
TILES = math.ceil(RPC / P)  # 49
GROUP_TILES = [6, 6, 6, 6, 6, 6, 6, 7]  # 8 groups over 49 tiles
CC_LAG = 2  # collective pieces trail the gather stream by this many groups


def _bf16(a):
    import ml_dtypes

    return np.asarray(a, dtype=np.float32).astype(ml_dtypes.bfloat16)


# ---------------------------------------------------------------- host prep
def preprocess(edge_index):
    """Graph-structure preprocessing (indices/degrees only)."""
    n = N_NODES
    src = np.asarray(edge_index[0], dtype=np.int64)
    dst = np.asarray(edge_index[1], dtype=np.int64)

    deg = np.bincount(dst, minlength=n).astype(np.float64) + 1.0  # + self loop
    dinv = 1.0 / np.sqrt(deg)
    rvec = dinv * np.bincount(dst, weights=dinv[src], minlength=n) + dinv * dinv

    core_of = dst // RPC
    local = dst - core_of * RPC
    tile_of = np.minimum(local // P, TILES - 1)
    # Table rows are laid out (group, core, local-row) so that per-group
    # AllGather pieces write contiguous ranges. Remap src node -> table row.
    gstart = np.concatenate([[0], np.cumsum(GROUP_TILES)])
    grow0 = gstart * P  # local row offset of each group (last entry clipped)
    rows_g = np.minimum(gstart[1:] * P, RPC) - grow0[:-1]
    base8 = np.concatenate([[0], np.cumsum(8 * rows_g)])
    group_of_tile = np.repeat(np.arange(len(GROUP_TILES)), GROUP_TILES)

    s_core = src // RPC
    s_local = src - s_core * RPC
    s_tile = np.minimum(s_local // P, TILES - 1)
    s_grp = group_of_tile[s_tile]
    rowpos = base8[s_grp] + s_core * rows_g[s_grp] + (s_local - grow0[s_grp])
    parity = rowpos & 1
    pidx = rowpos >> 1

    # chunk counts per (tile, parity), maxed over cores -> shared structure
    key = (core_of * TILES + tile_of) * 2 + parity
    cnt = np.bincount(key, minlength=N_CORES * TILES * 2).reshape(
        N_CORES, TILES, 2
    )
    CP = [
        [int(math.ceil(cnt[:, t, p].max() / P)) for p in (0, 1)]
        for t in range(TILES)
    ]
    ct = [CP[t][0] + CP[t][1] for t in range(TILES)]
    c0t = np.concatenate([[0], np.cumsum(ct)])  # global chunk start per tile
    c_tot = int(c0t[-1])
    chunk_parity = []
    for t in range(TILES):
        chunk_parity += [0] * CP[t][0] + [1] * CP[t][1]

    # groups / gather units
    assert gstart[-1] == TILES
    groups = []  # (tile0, ntiles, chunk0, nchunks)
    units = []  # (chunk0, nchunks, unit_of_group)
    for g in range(len(GROUP_TILES)):
        t0, t1 = int(gstart[g]), int(gstart[g + 1])
        groups.append((t0, t1 - t0, int(c0t[t0]), int(c0t[t1] - c0t[t0])))
        tm = t0 + (t1 - t0 + 1) // 2
        units.append((int(c0t[t0]), int(c0t[tm] - c0t[t0]), g))
        units.append((int(c0t[tm]), int(c0t[t1] - c0t[tm]), g))

    # sort edges by (core, tile, parity)
    order = np.argsort(key, kind="stable")
    pidx_s, local_s, key_s = pidx[order], local[order], key[order]
    bounds = np.searchsorted(key_s, np.arange(N_CORES * TILES * 2 + 1))

    per_core = []
    for c in range(N_CORES):
        idx_flat = np.zeros(c_tot * P, dtype=np.int16)
        dstl_flat = np.full(c_tot * P, -1.0, dtype=np.float32)
        pos = 0
        for t in range(TILES):
            for p in (0, 1):
                k = (c * TILES + t) * 2 + p
                a, b = bounds[k], bounds[k + 1]
                m = b - a
                cpad = CP[t][p] * P
                assert m <= cpad
                idx_flat[pos : pos + m] = pidx_s[a:b].astype(np.int16)
                dstl_flat[pos : pos + m] = (local_s[a:b] - t * P).astype(
                    np.float32
                )
                pos += cpad
        assert pos == c_tot * P

        idx_w = idx_flat.reshape(-1, 16).T.copy()  # [16, c_tot*8]
        idx_rep = np.tile(idx_w, (8, 1))  # [128, c_tot*8]
        dstl = dstl_flat.reshape(-1, P).T.copy()  # [128, c_tot]

        nodes = c * RPC + np.arange(TILES * P)
        valid = nodes < (c + 1) * RPC
        nodes_c = np.where(valid, nodes, c * RPC)
        col = lambda v: (
            np.where(valid, v[nodes_c], 0.0).reshape(TILES, P).T.astype(np.float32)
        ).copy()
        per_core.append(
            dict(
                idx=idx_rep,
                dstl=_bf16(dstl),
                cu0=col(1.0 / deg),
                cu0a=col(ALPHA / deg),
                cmid=col((1.0 - ALPHA) / deg),
                cfin=col(dinv),
                rcol=col(rvec),
            )
        )

    meta = dict(
        CP=CP,
        ct=ct,
        c0t=c0t,
        c_tot=c_tot,
        chunk_parity=chunk_parity,
        groups=groups,
        units=units,
        base8=[int(v) for v in base8],
    )
    return meta, per_core, dinv


def host_inputs(x, W1, b1, W3, b3, meta, per_core):
    W13 = (np.asarray(W1, np.float64) @ np.asarray(W3, np.float64)).astype(
        np.float32
    )
    b13 = (np.asarray(b1, np.float64) @ np.asarray(W3, np.float64)).astype(
        np.float32
    )
    b3 = np.asarray(b3, np.float32)
    iota = np.broadcast_to(np.arange(P, dtype=np.float32), (P, P)).copy()
    in_maps = []
    for c, pc in enumerate(per_core):
        xT = np.ascontiguousarray(np.asarray(x, np.float32)[c * RPC : (c + 1) * RPC].T)
        rb3 = (
            (pc["rcol"].reshape(P, TILES, 1) * b3.reshape(1, 1, F_OUT))
            .reshape(P, TILES * F_OUT)
            .astype(np.float32)
        )
        in_maps.append(
            dict(
                xT=_bf16(xT),
                W13=_bf16(W13),
                b13=b13.reshape(F_OUT, 1).copy(),
                rb3=rb3,
                idx=pc["idx"],
                dstl=pc["dstl"],
                cu0=pc["cu0"],
                cu0a=pc["cu0a"],
                cmid=pc["cmid"],
                cfin=pc["cfin"],
                iota=_bf16(iota),
            )
        )
    return in_maps


# ---------------------------------------------------------------- bass build
def build(meta):
    from concourse import bacc, mybir, tile
    from concourse.bass import AP
    from concourse import library_config
    from concourse.masks import make_identity

    f32 = mybir.dt.float32
    bf16 = mybir.dt.bfloat16
    i16 = mybir.dt.int16
    Copy = mybir.ActivationFunctionType.Copy
    Ident = mybir.ActivationFunctionType.Identity

    CP = meta["CP"]
    ct = meta["ct"]
    c0t = meta["c0t"]
    c_tot = meta["c_tot"]
    chunk_parity = meta["chunk_parity"]
    groups = meta["groups"]
    units = meta["units"]
    base8 = meta["base8"]
    n = N_NODES
    fo = F_OUT
    ctmax = max(ct)
    gcmax = max(g[3] for g in groups)
    rg = [list(range(N_CORES))]

    nc = bacc.Bacc(None, target_bir_lowering=False, debug=False, num_swdge_queues=4)

    xT_p = nc.declare_dram_parameter("xT", [F_IN, RPC], bf16, isOutput=False)
    W13_p = nc.declare_dram_parameter("W13", [F_IN, fo], bf16, isOutput=False)
    b13_p = nc.declare_dram_parameter("b13", [fo, 1], f32, isOutput=False)
    rb3_p = nc.declare_dram_parameter("rb3", [P, TILES * fo], f32, isOutput=False)
    idx_p = nc.declare_dram_parameter("idx", [P, c_tot * 8], i16, isOutput=False)
    dstl_p = nc.declare_dram_parameter("dstl", [P, c_tot], bf16, isOutput=False)
    cu0_p = nc.declare_dram_parameter("cu0", [P, TILES], f32, isOutput=False)
    cu0a_p = nc.declare_dram_parameter("cu0a", [P, TILES], f32, isOutput=False)
    cmid_p = nc.declare_dram_parameter("cmid", [P, TILES], f32, isOutput=False)
    cfin_p = nc.declare_dram_parameter("cfin", [P, TILES], f32, isOutput=False)
    iota_p = nc.declare_dram_parameter("iota", [P, P], bf16, isOutput=False)
    out_p = nc.declare_dram_parameter("out", [RPC, fo], f32, isOutput=True)

    # node tables (bf16, viewed as [n/2, 128] pair rows for the gather) and
    # per-round local shards
    T = [
        nc.dram_tensor(f"T{k}", [n, fo], bf16, addr_space="Shared")
        for k in range(N_ROUNDS)
    ]
    shard = [nc.dram_tensor(f"sh{k}", [RPC, fo], bf16) for k in range(N_ROUNDS)]

    qi = 0  # gather queue rotation counter

    with tile.TileContext(nc) as tc:
        nc.gpsimd.load_library(library_config.mlp)
        with (
            tc.tile_pool(name="const", bufs=1) as cp,
            tc.tile_pool(name="psA", bufs=2, space="PSUM") as psA,
            tc.tile_pool(name="psT", bufs=2, space="PSUM") as psT,
            tc.tile_pool(name="dense", bufs=3) as dp,
            tc.tile_pool(name="gat", bufs=4) as gp,
            tc.tile_pool(name="sel", bufs=4) as sp,
            tc.tile_pool(name="outt", bufs=4) as op,
            tc.tile_pool(name="tmpp", bufs=4) as tp_,
            tc.tile_pool(name="ps2", bufs=4, space="PSUM") as pp2,
        ):
            # ---------------- resident constants ----------------
            idx_sb = cp.tile([P, c_tot * 8], i16)
            nc.sync.dma_start(idx_sb[:], idx_p[:])
            dstl_sb = cp.tile([P, c_tot], bf16)
            nc.sync.dma_start(dstl_sb[:], dstl_p[:])
            cu0_sb = cp.tile([P, TILES], f32)
            nc.sync.dma_start(cu0_sb[:], cu0_p[:])
            cu0a_sb = cp.tile([P, TILES], f32)
            nc.sync.dma_start(cu0a_sb[:], cu0a_p[:])
            cmid_sb = cp.tile([P, TILES], f32)
            nc.sync.dma_start(cmid_sb[:], cmid_p[:])
            cfin_sb = cp.tile([P, TILES], f32)
            nc.sync.dma_start(cfin_sb[:], cfin_p[:])
            iota_sb = cp.tile([P, P], bf16)
            nc.sync.dma_start(iota_sb[:], iota_p[:])
            rb3_sb = cp.tile([P, TILES * fo], f32)
            nc.sync.dma_start(rb3_sb[:], rb3_p[:])
            b13_sb = cp.tile([fo, 1], f32)
            nc.sync.dma_start(b13_sb[:], b13_p[:])
            w13_sb = cp.tile([P, (F_IN // P) * fo], bf16)
            for kk in range(F_IN // P):
                nc.sync.dma_start(
                    w13_sb[:, kk * fo : (kk + 1) * fo],
                    W13_p[kk * P : (kk + 1) * P, :],
                )
            ident = cp.tile([P, P], f32)
            make_identity(nc, ident[:])
            # node-major local state, one 64-col slot per tile
            z_sb = cp.tile([P, TILES * fo], bf16)
            uprev_sb = cp.tile([P, TILES * fo], bf16)
            u0s_sb = cp.tile([P, TILES * fo], bf16)

            # ---------------- dense phase: z = dinv*(x@W13 + b13) ----------
            NBLK = 512
            nblocks = math.ceil(RPC / NBLK)
            for bi in range(nblocks):
                w = min(NBLK, RPC - bi * NBLK)
                ps = psA.tile([fo, NBLK], f32, tag="ps")
                for kk in range(F_IN // P):
                    xt = dp.tile([P, NBLK], bf16, tag="xt")
                    nc.sync.dma_start(
                        xt[:, :w],
                        xT_p[kk * P : (kk + 1) * P, bi * NBLK : bi * NBLK + w],
                    )
                    nc.tensor.matmul(
                        ps[:, :w],
                        lhsT=w13_sb[:, kk * fo : (kk + 1) * fo],
                        rhs=xt[:, :w],
                        start=(kk == 0),
                        stop=(kk == F_IN // P - 1),
                    )
                zt = dp.tile([fo, NBLK], f32, tag="zt")
                nc.scalar.activation(zt[:, :w], ps[:, :w], Ident, bias=b13_sb[:, :1])
                for s in range(math.ceil(w / P)):
                    sw = min(P, w - s * P)
                    t = (bi * NBLK + s * P) // P  # global tile index
                    pt = psT.tile([P, fo], f32, tag="pt")
                    nc.tensor.transpose(
                        pt[:sw, :], zt[:, s * P : s * P + sw], ident[:fo, :fo]
                    )
                    nc.scalar.activation(
                        z_sb[:sw, t * fo : t * fo + fo],
                        pt[:sw, :],
                        Copy,
                        scale=cfin_sb[:sw, t : t + 1],
                    )
                    nc.sync.dma_start(
                        shard[0][t * P : t * P + sw, :],
                        z_sb[:sw, t * fo : t * fo + fo],
                    )
            for g, (t0, ntiles, _, _) in enumerate(groups):
                r0 = t0 * P
                r1 = min((t0 + ntiles) * P, RPC)
                nc.gpsimd.collective_compute(
                    "AllGather",
                    mybir.AluOpType.bypass,
                    replica_groups=rg,
                    ins=[shard[0][r0:r1, :]],
                    outs=[T[0][base8[g] : base8[g] + 8 * (r1 - r0), :]],
                )

            # ---------------- propagation rounds ----------------
            # strict rotation: each q0 gather blocks the engine for its own
            # generation, during which the three async queues generate in
            # parallel -> 4-way descriptor generation at steady state
            qsched = [ui % 4 for ui in range(len(units))]

            def emit_cc(rnd, g):
                t0, ntiles, _, _ = groups[g]
                r0 = t0 * P
                r1 = min((t0 + ntiles) * P, RPC)
                nc.gpsimd.collective_compute(
                    "AllGather",
                    mybir.AluOpType.bypass,
                    replica_groups=rg,
                    ins=[shard[rnd + 1][r0:r1, :]],
                    outs=[T[rnd + 1][base8[g] : base8[g] + 8 * (r1 - r0), :]],
                )

            for rnd in range(N_ROUNDS):
                Tin = T[rnd][:, :].rearrange("(a b) e -> a (b e)", b=2)
                for g, (t0, ntiles, gc0, gnc) in enumerate(groups):
                    gb = gp.tile([P, gcmax * P], bf16, tag="gb")
                    for u in range(2):
                        uc0, unc, _ = units[g * 2 + u]
                        if unc == 0:
                            continue
                        nc.gpsimd.dma_gather(
                            out_ap=gb[
                                :, (uc0 - gc0) * P : (uc0 - gc0 + unc) * P
                            ].rearrange("p (c e) -> p c e", e=P),
                            in_ap=Tin,
                            idxs_ap=idx_sb[:, uc0 * 8 : (uc0 + unc) * 8],
                            num_idxs=unc * P,
                            num_idxs_reg=unc * P,
                            elem_size=P,
                            single_packet=False,
                            queue_num=qsched[g * 2 + u],
                        )
                    if rnd < N_ROUNDS - 1 and g - CC_LAG >= 0:
                        emit_cc(rnd, g - CC_LAG)
                    for tl in range(ntiles):
                        t = t0 + tl
                        rows = min(P, RPC - t * P)
                        tc0 = int(c0t[t])
                        tct = ct[t]
                        # selector build: S[p, c, r] = (dstl[p, c] == r)
                        st = sp.tile([P, ctmax * P], bf16, tag="st")
                        din = dstl_sb[:, tc0 : tc0 + tct].to_broadcast(
                            [P, tct, P]
                        )
                        iin = AP(
                            iota_sb[:].tensor,
                            iota_sb[:].offset,
                            [iota_sb[:].ap[0], [0, tct], [1, P]],
                        )
                        sout = st[:, 0 : tct * P].rearrange(
                            "p (c r) -> p c r", r=P
                        )
                        nc.vector.tensor_tensor(
                            out=sout, in0=din, in1=iin, op=mybir.AluOpType.is_equal
                        )
                        pt = pp2.tile([P, fo], f32, tag="pt2")
                        for j in range(tct):
                            gcol = (tc0 - gc0 + j) * P + chunk_parity[tc0 + j] * fo
                            nc.tensor.matmul(
                                pt[:, :],
                                lhsT=st[:, j * P : (j + 1) * P],
                                rhs=gb[:, gcol : gcol + fo],
                                start=(j == 0),
                                stop=(j == tct - 1),
                            )
                        # epilogue: self-loop add + per-node scaling
                        so = t * fo
                        tmp = tp_.tile([P, fo], f32, tag="tmp")
                        if rnd == 0:
                            nc.vector.tensor_add(
                                tmp[:rows, :], pt[:rows, :], z_sb[:rows, so : so + fo]
                            )
                            nc.scalar.activation(
                                uprev_sb[:rows, so : so + fo],
                                tmp[:rows, :],
                                Copy,
                                scale=cu0_sb[:rows, t : t + 1],
                            )
                            nc.scalar.activation(
                                u0s_sb[:rows, so : so + fo],
                                tmp[:rows, :],
                                Copy,
                                scale=cu0a_sb[:rows, t : t + 1],
                            )
                            nc.sync.dma_start(
                                shard[1][t * P : t * P + rows, :],
                                uprev_sb[:rows, so : so + fo],
                            )
                        elif rnd < N_ROUNDS - 1:
                            nc.vector.tensor_add(
                                tmp[:rows, :],
                                pt[:rows, :],
                                uprev_sb[:rows, so : so + fo],
                            )
                            tmp2 = tp_.tile([P, fo], f32, tag="tmp2")
                            nc.scalar.activation(
                                tmp2[:rows, :],
                                tmp[:rows, :],
                                Copy,
                                scale=cmid_sb[:rows, t : t + 1],
                            )
                            nc.vector.tensor_add(
                                uprev_sb[:rows, so : so + fo],
                                tmp2[:rows, :],
                                u0s_sb[:rows, so : so + fo],
                            )
                            nc.sync.dma_start(
                                shard[rnd + 1][t * P : t * P + rows, :],
                                uprev_sb[:rows, so : so + fo],
                            )
                        else:
                            nc.vector.tensor_add(
                                tmp[:rows, :],
                                pt[:rows, :],
                                uprev_sb[:rows, so : so + fo],
                            )
                            tmp2 = tp_.tile([P, fo], f32, tag="tmp2")
                            nc.scalar.activation(
                                tmp2[:rows, :],
                                tmp[:rows, :],
                                Copy,
                                scale=cfin_sb[:rows, t : t + 1],
                            )
                            ot = op.tile([P, fo], f32, tag="ot")
                            nc.vector.tensor_add(
                                ot[:rows, :],
                                tmp2[:rows, :],
                                rb3_sb[:rows, so : so + fo],
                            )
                            nc.sync.dma_start(
                                out_p[t * P : t * P + rows, :], ot[:rows, :]
                            )
                # trailing collective pieces for the last groups
                if rnd < N_ROUNDS - 1:
                    for g in range(len(groups) - CC_LAG, len(groups)):
                        emit_cc(rnd, g)
    nc.compile()
    return nc


# ---------------------------------------------------------------- runner
def run(x, edge_index, W1, b1, W3, b3, trace=False):
    from concourse.bass_utils import run_bass_kernel_spmd

    meta, per_core, _ = preprocess(edge_index)
    in_maps = host_inputs(x, W1, b1, W3, b3, meta, per_core)
    nc = build(meta)
    res = run_bass_kernel_spmd(
        nc, in_maps, core_ids=list(range(N_CORES)), trace=trace
    )
    out = np.concatenate(
        [res.results[i]["out"] for i in range(N_CORES)], axis=0
    )
    return out.astype(np.float32), res


def kernel(**inputs):
    out, _ = run(
        inputs["x"],
        inputs["edge_index"],
        inputs["W1"],
        inputs["b1"],
        inputs["W3"],
        inputs["b3"],
    )
    return out
